# revision 1
# baseline (speedup 1.0000x reference)
"""Trainium2 Bass kernel v2 for nn_AdaptiveGNN (4-layer GCN, N=100000,
E=1600000, dims 128->256->256->256->128), 8-core node-sharded.

v2 strategy vs v1: dense slot packing (no fixed-k padding), transpose=False
dma_gather on 4 SWDGE queues (slots land on partitions), segment sums done on
the TensorEngine via host-built 0/1 S matrices (shared by all 4 layers),
accumulated in f32 PSUM per 512-destination bank; self-loops injected via
sequential transposed DMA of the core's own rows; chunked AllGathers overlap
the tail of each layer's compute.
"""
import sys
for p in ("/opt/trn_rl_repo",):
    if p not in sys.path:
        sys.path.insert(0, p)
import numpy as np
import ml_dtypes

from concourse import bass, bacc, mybir, tile
from concourse.library_config import mlp
from concourse.masks import make_identity

_bf16 = ml_dtypes.bfloat16

N = 100000
E = 1600000
C = 8
NPC = N // C              # 12500 real nodes per core
NPCP = 12544              # padded rows per core (98 blocks of 128)
NPT = C * NPCP            # 100352 table rows
BANK = 512                # destinations per PSUM bank tile
NBANK = (NPCP + BANK - 1) // BANK   # 25 (last bank 256)
SPLIT = 6144              # local rows per core in table chunk A (12 banks)
SPLA = C * SPLIT          # 49152 rows in chunk A
SPLB = NPCP - SPLIT       # 6400 local rows in chunk B
NW = 4                    # int16 windows covering the chunk-major row space
WBASE = [0, 32768, SPLA, SPLA + 32768]
WEND = [32768, SPLA, SPLA + 32768, NPT]
NQ = 4                    # SWDGE queues
CHUNK = 896               # max slots per dma_gather (SWDGE ring limit)
GBUFS = 12


def row_of_node(n):
    c = n // NPC
    l = n % NPC
    return np.where(l < SPLIT, c * SPLIT + l, SPLA + c * SPLB + (l - SPLIT))

dt = mybir.dt
F32, BF16, I16 = dt.float32, dt.bfloat16, dt.int16
AX = mybir.AxisListType
OP = mybir.AluOpType
AF = mybir.ActivationFunctionType
LN_EPS = 1e-5
RG = [list(range(C))]


def preprocess(edge_index: np.ndarray):
    """Build per-core slot streams, S matrices, and uniform chunk metadata."""
    src0 = edge_index[0].astype(np.int64)
    dst0 = edge_index[1].astype(np.int64)

    deg = np.bincount(dst0, minlength=N).astype(np.int64) + 1
    dinv = (1.0 / np.sqrt(deg.astype(np.float64))).astype(np.float32)

    core_of = dst0 // NPC
    d_loc = dst0 % NPC                       # local dst in [0, NPC)
    src_r = row_of_node(src0)                # chunk-major global row
    w = ((src_r >= WBASE[1]).astype(np.int64)
         + (src_r >= WBASE[2]) + (src_r >= WBASE[3]))
    idxv = src_r - np.asarray(WBASE)[w]
    bank = d_loc // BANK

    # per (core, bank, window) counts
    nn = np.zeros((C, NBANK, NW), np.int64)
    key = ((core_of * NBANK + bank) * NW + w)
    np.add.at(nn.reshape(-1), key, 1)
    nn_max = nn.max(axis=0)                  # [NBANK, NW]

    # sort edges by (core, bank, w, d)
    order = np.argsort(key * NPCP + d_loc, kind="stable")
    s_srt = idxv[order]
    d_srt = d_loc[order]
    k_srt = key[order]
    # start offset of each (c,b,w) run in the sorted arrays
    run_starts = np.zeros(C * NBANK * NW + 1, np.int64)
    np.cumsum(np.bincount(k_srt, minlength=C * NBANK * NW),
              out=run_starts[1:])

    # ---- uniform chunk layout (shared across cores) ----
    banks_meta = []
    off16 = 0        # stream offset in 16-slot units
    scols = 0        # S column offset
    slot_base = np.zeros((NBANK, NW), np.int64)  # global padded slot offset
    for b in range(NBANK):
        nb_d = min(BANK, NPCP - b * BANK)
        chunks = []
        for wi in range(NW):
            m = int(nn_max[b, wi])
            slot_base[b, wi] = off16 * 16
            pos = 0
            while pos < m:
                size = min(CHUNK, m - pos)
                ngrp = (size + 127) // 128
                groups = [dict(gi=gi, s_off=0, c0=0, ncols=0,
                               first=False, last=False)
                          for gi in range(ngrp)]
                chunks.append(dict(w=wi, off16=off16, nn=size, groups=groups,
                                   slot0=off16 * 16))
                off16 += (size + 15) // 16
                pos += size
        assert chunks, f"bank {b} has no slots"
        # mark first/last matmul of the bank
        chunks[0]["groups"][0]["first"] = True
        chunks[-1]["groups"][-1]["last"] = True
        banks_meta.append(dict(nb_d=nb_d, chunks=chunks))
    ICOLS = off16

    # ---- per-core slot arrays (idx + d per padded slot) ----
    TOTS = ICOLS * 16
    idx_all = np.zeros((C, TOTS), np.int16)
    dcol_all = np.full((C, TOTS), -1, np.int64)   # local d per slot, -1 = pad
    for c in range(C):
        for b in range(NBANK):
            for wi in range(NW):
                r = (c * NBANK + b) * NW + wi
                lo, hi = run_starts[r], run_starts[r + 1]
                cnt = hi - lo
                base = slot_base[b, wi]
                idx_all[c, base:base + cnt] = s_srt[lo:hi]
                dcol_all[c, base:base + cnt] = d_srt[lo:hi] - b * BANK

    # ---- group col ranges (min/max over cores) + S fill ----
    # first pass: compute c0/ncols
    for b, bm in enumerate(banks_meta):
        nb_d = bm["nb_d"]
        for ch in bm["chunks"]:
            s0 = ch["slot0"]
            for g in ch["groups"]:
                a = s0 + g["gi"] * 128
                e = min(s0 + ch["nn"], a + 128)
                dv = dcol_all[:, a:e]
                valid = dv >= 0
                if g["first"]:
                    g["c0"], g["ncols"] = 0, nb_d
                elif valid.any():
                    g["c0"] = int(dv[valid].min())
                    g["ncols"] = int(dv[valid].max()) - g["c0"] + 1
                else:
                    g["c0"], g["ncols"] = 0, 1
    # assign S offsets and total size (bank starts 16-aligned)
    scols = 0
    for bm in banks_meta:
        scols = (scols + 15) // 16 * 16
        bm["s_off"] = scols
        for ch in bm["chunks"]:
            for g in ch["groups"]:
                g["s_off"] = scols
                scols += g["ncols"]
        bm["s_cols"] = scols - bm["s_off"]
    SCOLS = (scols + 15) // 16 * 16

    S = np.zeros((C, 128, SCOLS), np.float32)
    for b, bm in enumerate(banks_meta):
        for ch in bm["chunks"]:
            s0 = ch["slot0"]
            for g in ch["groups"]:
                a = s0 + g["gi"] * 128
                e = min(s0 + ch["nn"], a + 128)
                dv = dcol_all[:, a:e]           # [C, <=128]
                valid = dv >= 0
                ci, pi = np.nonzero(valid)
                cols = dv[ci, pi] - g["c0"]
                np.add.at(S, (ci, pi, g["s_off"] + cols), 1.0)

    # ---- aux host data (chunk-major row space) ----
    rows = np.arange(NPT)
    in_a = rows < SPLA
    c_r = np.where(in_a, rows // SPLIT, (rows - SPLA) // SPLB)
    l_r = np.where(in_a, rows % SPLIT, SPLIT + (rows - SPLA) % SPLB)
    row_real = l_r < NPC
    node_of_row = c_r * NPC + np.minimum(l_r, NPC - 1)
    dinv_rows = np.zeros(NPT, np.float32)
    dinv_rows[row_real] = dinv[node_of_row[row_real]]

    # per-core local-order dinv [C, NPCP]
    dloc = np.zeros((C, NPCP), np.float32)
    lr = np.arange(NPC)
    for c in range(C):
        dloc[c, :NPC] = dinv[c * NPC + lr]
    dinv_blocks = dloc.reshape(C, NPCP // 128, 128).transpose(0, 2, 1)
    dinv_frow = np.zeros((C, NBANK, BANK), np.float32)
    for b in range(NBANK):
        nb_d = min(BANK, NPCP - b * BANK)
        dinv_frow[:, b, :nb_d] = dloc[:, b * BANK:b * BANK + nb_d]

    return dict(deg=deg, dinv=dinv, dinv_rows=dinv_rows, dloc=dloc,
                dinv_blocks=np.ascontiguousarray(dinv_blocks),
                dinv_frow=dinv_frow, row_real=row_real,
                node_of_row=node_of_row,
                banks=banks_meta, ICOLS=ICOLS, SCOLS=SCOLS,
                idx=idx_all, S=S.astype(_bf16))


def pack_idx(enc_core: np.ndarray) -> np.ndarray:
    """[TOTS] int16 -> [128, TOTS//16], slot i at (i%16, i//16), tiled x8."""
    TOTS = len(enc_core)
    block = enc_core.reshape(TOTS // 16, 16).T
    return np.tile(block, (8, 1)).copy()


def build(pp, variant="full"):
    banks = pp["banks"]
    ICOLS = pp["ICOLS"]
    SCOLS = pp["SCOLS"]
    max_scols = max(bm["s_cols"] for bm in banks)
    max_scols = (max_scols + 15) // 16 * 16

    nc = bacc.Bacc("TRN2", target_bir_lowering=False, debug=False,
                   num_devices=C, num_swdge_queues=NQ)

    t0 = nc.dram_tensor("t0", [NPT, 128], BF16, kind="ExternalInput")
    t0_own = nc.dram_tensor("t0own", [NPCP, 128], BF16, kind="ExternalInput")
    idx = nc.dram_tensor("idx", [128, ICOLS], I16, kind="ExternalInput")
    s_in = nc.dram_tensor("smat", [128, SCOLS], BF16, kind="ExternalInput")
    dinv_in = nc.dram_tensor("dinv", [128, NPCP // 128], F32,
                             kind="ExternalInput")
    dfrow_in = nc.dram_tensor("dfrow", [NBANK, BANK], F32,
                              kind="ExternalInput")
    b3col_in = nc.dram_tensor("b3col", [128, 1], F32, kind="ExternalInput")
    w0_in = nc.dram_tensor("w0", [128, 256], BF16, kind="ExternalInput")
    w1_in = nc.dram_tensor("w1", [256, 256], BF16, kind="ExternalInput")
    w2_in = nc.dram_tensor("w2", [256, 256], BF16, kind="ExternalInput")
    w3_in = nc.dram_tensor("w3", [256, 128], BF16, kind="ExternalInput")
    # rows 0..2 = b0,b1,b2; 3..5 = g0,g1,g2; 6..8 = beta0..2
    vecs = nc.dram_tensor("vecs", [9, 256], F32, kind="ExternalInput")
    out = nc.dram_tensor("out", [128, NPCP], F32, kind="ExternalOutput")

    with tile.TileContext(nc) as tc:
        nc.gpsimd.load_library(mlp)
        with (tc.tile_pool(name="dram", bufs=1, space="DRAM") as dr,
              tc.tile_pool(name="const", bufs=1) as cp,
              tc.tile_pool(name="sp", bufs=3) as spl,
              tc.tile_pool(name="ap", bufs=2) as apl,
              tc.tile_pool(name="ep", bufs=2) as ep,
              tc.tile_pool(name="hp", bufs=2) as hp,
              tc.tile_pool(name="pp", bufs=2, space="PSUM") as pps,
              tc.tile_pool(name="pw", bufs=2, space="PSUM") as pwp,
              tc.tile_pool(name="pz", bufs=1, space="PSUM") as pzp):

            ag1a = dr.tile([SPLIT, 256], BF16)
            ag1b = dr.tile([SPLB, 256], BF16)
            t1a = dr.tile([SPLA, 256], BF16, addr_space="Shared")
            t1b = dr.tile([C * SPLB, 256], BF16, addr_space="Shared")
            ag2a = dr.tile([SPLIT, 256], BF16)
            ag2b = dr.tile([SPLB, 256], BF16)
            t2a = dr.tile([SPLA, 256], BF16, addr_space="Shared")
            t2b = dr.tile([C * SPLB, 256], BF16, addr_space="Shared")
            agza = dr.tile([SPLIT, 128], BF16)
            agzb = dr.tile([SPLB, 128], BF16)
            tza = dr.tile([SPLA, 128], BF16, addr_space="Shared")
            tzb = dr.tile([C * SPLB, 128], BF16, addr_space="Shared")

            # ---- constants ----
            idx_t = cp.tile([128, ICOLS], I16)
            nc.sync.dma_start(out=idx_t[:], in_=idx[:])
            dinv_sb = cp.tile([128, NPCP // 128], F32)
            nc.sync.dma_start(out=dinv_sb[:], in_=dinv_in[:])
            b3c = cp.tile([128, 1], F32)
            nc.sync.dma_start(out=b3c[:], in_=b3col_in[:])
            w0_sb = cp.tile([128, 1, 256], BF16)
            nc.sync.dma_start(out=w0_sb[:],
                              in_=w0_in[:].rearrange("(j p) o -> p j o", p=128))
            w1_sb = cp.tile([128, 2, 256], BF16)
            nc.sync.dma_start(out=w1_sb[:],
                              in_=w1_in[:].rearrange("(j p) o -> p j o", p=128))
            w2_sb = cp.tile([128, 2, 256], BF16)
            nc.sync.dma_start(out=w2_sb[:],
                              in_=w2_in[:].rearrange("(j p) o -> p j o", p=128))
            w3_sb = cp.tile([128, 2, 128], BF16)
            nc.sync.dma_start(out=w3_sb[:],
                              in_=w3_in[:].rearrange("(j p) o -> p j o", p=128))

            vec_row = cp.tile([1, 9 * 256], F32)
            nc.sync.dma_start(out=vec_row[:], in_=vecs[:].flatten().unsqueeze(0))
            bcast = cp.tile([128, 9, 256], F32)
            for r in range(9):
                nc.gpsimd.partition_broadcast(
                    out_ap=bcast[:, r, :],
                    in_ap=vec_row[:, r * 256:(r + 1) * 256])

            ident = cp.tile([128, 128], BF16)
            make_identity(nc, ident[:])
            eps_t = cp.tile([128, 1], F32)
            nc.vector.memset(eps_t[:], LN_EPS)

            # G buffers: manual rotation, memset once (kills stale NaN risk)
            gbig = cp.tile([128, GBUFS, 7 * 256], BF16)
            nc.vector.memset(gbig[:], 0.0)

            def vbc(r, nb, d=256):
                return bcast[:, r:r + 1, :d].to_broadcast([128, nb, d])

            def wbases(ta, tb):
                return [(ta, 0), (ta, 32768), (tb, 0), (tb, 32768)]

            layers = [
                dict(wins=[(t0, WBASE[i]) for i in range(NW)], Din=128,
                     w=w0_sb, bias=0, g=3, beta=6,
                     ag=(ag1a, ag1b), nxt=(t1a, t1b), own=(t0_own, None)),
                dict(wins=wbases(t1a, t1b), Din=256,
                     w=w1_sb, bias=1, g=4, beta=7,
                     ag=(ag2a, ag2b), nxt=(t2a, t2b), own=(ag1a, ag1b)),
                dict(wins=wbases(t2a, t2b), Din=256,
                     w=w2_sb, bias=2, g=5, beta=8,
                     ag=(agza, agzb), nxt=(tza, tzb), own=(ag2a, ag2b),
                     with_z=True),
                dict(wins=wbases(tza, tzb), Din=128, w=None,
                     own=(agza, agzb)),
            ]

            grot = [0]

            def do_layer(li, L):
                Din = L["Din"]
                J = Din // 128
                is_final = L["w"] is None

                ag_cut = SPLIT // BANK   # banks 0..11 are table chunk A
                for b, bm in enumerate(banks):
                    nb_d = bm["nb_d"]
                    ncr = nb_d // 128       # dst chunks of 128

                    s_t = spl.tile([128, max_scols], BF16, tag="S",
                                   name=f"S_{li}_{b}")
                    sc16 = (bm["s_cols"] + 15) // 16 * 16
                    nc.sync.dma_start(
                        out=s_t[:, :sc16],
                        in_=s_in[:, bm["s_off"]:bm["s_off"] + sc16])
                    soff0 = bm["s_off"]

                    pts = [pps.tile([128, BANK], F32, tag=f"agg{j}",
                                    name=f"pt{j}_{li}_{b}", space="PSUM")
                           for j in range(J)]

                    for ch in bm["chunks"]:
                        nn_c = ch["nn"]
                        ngr = (nn_c + 127) // 128
                        gslot = grot[0] % GBUFS
                        grot[0] += 1
                        gt = gbig[:, gslot, :ngr * Din].rearrange(
                            "p (n e) -> p n e", e=Din)
                        wt, woff = L["wins"][ch["w"]]
                        nc.gpsimd.dma_gather(
                            out_ap=gt,
                            in_ap=wt[woff:, :],
                            idxs_ap=idx_t[:, ch["off16"]:
                                          ch["off16"] + (nn_c + 15) // 16],
                            num_idxs=nn_c, num_idxs_reg=nn_c, elem_size=Din,
                            transpose=False, queue_num=grot[0] % NQ,
                            single_packet=(Din == 128),
                        )
                        if variant == "gather_only":
                            continue
                        for g in ch["groups"]:
                            rel = g["s_off"] - soff0
                            for j in range(J):
                                nc.tensor.matmul(
                                    out=pts[j][:, g["c0"]:g["c0"] + g["ncols"]],
                                    lhsT=gbig[:, gslot,
                                              g["gi"] * Din + j * 128:
                                              g["gi"] * Din + j * 128 + 128],
                                    rhs=s_t[:, rel:rel + g["ncols"]],
                                    start=g["first"], stop=g["last"],
                                    skip_group_check=True)

                    if variant == "gather_only":
                        continue
                    # self rows (transposed sequential DMA) + psum -> sbuf
                    own_a, own_b = L["own"]
                    if own_b is None:
                        own_t, orow = own_a, b * BANK
                    elif b < ag_cut:
                        own_t, orow = own_a, b * BANK
                    else:
                        own_t, orow = own_b, b * BANK - SPLIT
                    selfT = apl.tile([128, J, BANK], BF16, tag="selfT",
                                     name=f"sf_{li}_{b}")
                    for j in range(J):
                        nc.sync.dma_start_transpose(
                            out=selfT[:, j, :nb_d],
                            in_=own_t[orow:orow + nb_d,
                                      j * 128:(j + 1) * 128])
                    asb = apl.tile([128, J, BANK], BF16, tag="asb",
                                   name=f"as_{li}_{b}")
                    for j in range(J):
                        nc.vector.tensor_tensor(
                            out=asb[:, j, :nb_d], in0=pts[j][:, :nb_d],
                            in1=selfT[:, j, :nb_d], op=OP.add)

                    if is_final:
                        # out[f, d] = dinv_d * asb + b3[f]
                        dbc = ep.tile([128, BANK], F32, tag="dbc",
                                      name=f"db_{b}")
                        drow = ep.tile([1, BANK], F32, tag="drow",
                                       name=f"dr_{b}")
                        nc.sync.dma_start(out=drow[:], in_=dfrow_in[b:b + 1, :])
                        nc.gpsimd.partition_broadcast(out_ap=dbc[:],
                                                      in_ap=drow[:])
                        ob = ep.tile([128, BANK], F32, tag="ob", name=f"ob_{b}")
                        nc.vector.tensor_tensor(
                            out=ob[:, :nb_d], in0=asb[:, 0, :nb_d],
                            in1=dbc[:, :nb_d], op=OP.mult)
                        nc.vector.tensor_scalar(
                            out=ob[:, :nb_d], in0=ob[:, :nb_d],
                            scalar1=b3c[:, :1], scalar2=None, op0=OP.add)
                        nc.sync.dma_start(
                            out=out[:, b * BANK:b * BANK + nb_d],
                            in_=ob[:, :nb_d])
                        continue

                    # W matmul per 128-dst chunk -> cs row-major
                    cs = ep.tile([128, ncr, 256], F32, tag="cs",
                                 name=f"cs_{li}_{b}")
                    for k in range(ncr):
                        pwt = pwp.tile([128, 256], F32, tag="pw",
                                       name=f"pw_{li}_{b}_{k}", space="PSUM")
                        for j in range(J):
                            nc.tensor.matmul(
                                out=pwt[:],
                                lhsT=asb[:, j, k * 128:(k + 1) * 128],
                                rhs=L["w"][:, j, :],
                                start=(j == 0), stop=(j == J - 1))
                        nc.vector.tensor_scalar(
                            out=cs[:, k, :], in0=pwt[:],
                            scalar1=dinv_sb[:, b * 4 + k:b * 4 + k + 1],
                            scalar2=None, op0=OP.mult)
                    nc.vector.tensor_tensor(
                        out=cs[:, :ncr, :], in0=cs[:, :ncr, :],
                        in1=vbc(L["bias"], ncr), op=OP.add)

                    # LayerNorm over feature dim (256)
                    mu = ep.tile([128, ncr], F32, tag="mu", name=f"mu_{li}_{b}")
                    with nc.allow_low_precision("LN mean"):
                        nc.vector.tensor_reduce(out=mu[:, :ncr],
                                                in_=cs[:, :ncr, :],
                                                axis=AX.X, op=OP.add)
                    nc.scalar.mul(out=mu[:, :ncr], in_=mu[:, :ncr],
                                  mul=1.0 / 256.0)
                    nc.vector.tensor_tensor(
                        out=cs[:, :ncr, :], in0=cs[:, :ncr, :],
                        in1=mu[:, :ncr].unsqueeze(2).to_broadcast(
                            [128, ncr, 256]),
                        op=OP.subtract)
                    sq = ep.tile([128, ncr, 256], F32, tag="sq",
                                 name=f"sq_{li}_{b}")
                    nc.scalar.square(out=sq[:, :ncr, :], in_=cs[:, :ncr, :])
                    var = ep.tile([128, ncr], F32, tag="var",
                                  name=f"v_{li}_{b}")
                    with nc.allow_low_precision("LN var"):
                        nc.vector.tensor_reduce(out=var[:, :ncr],
                                                in_=sq[:, :ncr, :],
                                                axis=AX.X, op=OP.add)
                    nc.scalar.activation(out=var[:, :ncr], in_=var[:, :ncr],
                                         func=AF.Sqrt, scale=1.0 / 256.0,
                                         bias=eps_t[:, :1])
                    nc.vector.reciprocal(out=var[:, :ncr], in_=var[:, :ncr])
                    for k in range(ncr):
                        nc.scalar.mul(out=cs[:, k, :], in_=cs[:, k, :],
                                      mul=var[:, k:k + 1])
                    nc.vector.tensor_tensor(
                        out=cs[:, :ncr, :], in0=cs[:, :ncr, :],
                        in1=vbc(L["g"], ncr), op=OP.mult)
                    nc.vector.tensor_tensor(
                        out=cs[:, :ncr, :], in0=cs[:, :ncr, :],
                        in1=vbc(L["beta"], ncr), op=OP.add)

                    # ELU, then * dinv -> bf16 table rows
                    amx = ep.tile([128, ncr, 256], F32, tag="amx",
                                  name=f"a_{li}_{b}")
                    nc.vector.tensor_scalar_max(out=amx[:, :ncr, :],
                                                in0=cs[:, :ncr, :], scalar1=0.0)
                    nc.vector.tensor_scalar_min(out=cs[:, :ncr, :],
                                                in0=cs[:, :ncr, :], scalar1=0.0)
                    nc.scalar.activation(out=cs[:, :ncr, :], in_=cs[:, :ncr, :],
                                         func=AF.Exp)
                    nc.vector.tensor_tensor(out=cs[:, :ncr, :],
                                            in0=cs[:, :ncr, :],
                                            in1=amx[:, :ncr, :], op=OP.add)
                    nc.vector.tensor_scalar_add(out=cs[:, :ncr, :],
                                                in0=cs[:, :ncr, :],
                                                scalar1=-1.0)
                    hh = hp.tile([128, ncr, 256], BF16, tag="hh",
                                 name=f"h_{li}_{b}")
                    for k in range(ncr):
                        nc.vector.tensor_scalar(
                            out=hh[:, k, :], in0=cs[:, k, :],
                            scalar1=dinv_sb[:, b * 4 + k:b * 4 + k + 1],
                            scalar2=None, op0=OP.mult)
                    if not L.get("with_z"):
                        agt = L["ag"][0] if b < ag_cut else L["ag"][1]
                        arow = b * BANK if b < ag_cut else b * BANK - SPLIT
                        nc.sync.dma_start(
                            out=agt[arow:arow + nb_d, :]
                                .rearrange("(nb p) d -> p nb d", p=128),
                            in_=hh[:, :ncr, :])
                    else:
                        # z = hh @ W3 -> agz (hh itself is never a table)
                        zc = hp.tile([128, ncr, 128], BF16, tag="zc",
                                     name=f"zc_{b}")
                        for k in range(ncr):
                            h3T = hp.tile([128, 2, 128], BF16, tag="h3T",
                                          name=f"h3T_{b}_{k}")
                            for j in range(2):
                                pt2 = pzp.tile([128, 128], BF16, tag="ptz",
                                               name=f"ptz_{b}_{k}_{j}",
                                               space="PSUM")
                                nc.tensor.transpose(
                                    out=pt2[:],
                                    in_=hh[:, k, j * 128:(j + 1) * 128],
                                    identity=ident[:])
                                nc.vector.tensor_copy(out=h3T[:, j, :],
                                                      in_=pt2[:])
                            pz = pzp.tile([128, 128], F32, tag="pz",
                                          name=f"pz_{b}_{k}", space="PSUM")
                            for j in range(2):
                                nc.tensor.matmul(
                                    out=pz[:], lhsT=h3T[:, j, :],
                                    rhs=w3_sb[:, j, :],
                                    start=(j == 0), stop=(j == 1))
                            nc.vector.tensor_copy(out=zc[:, k, :], in_=pz[:])
                        agt = L["ag"][0] if b < ag_cut else L["ag"][1]
                        arow = b * BANK if b < ag_cut else b * BANK - SPLIT
                        nc.sync.dma_start(
                            out=agt[arow:arow + nb_d, :]
                                .rearrange("(nb p) d -> p nb d", p=128),
                            in_=zc[:, :ncr, :])

                    # fire first half collective early
                    if b == ag_cut - 1 and not is_final and \
                            variant != "no_coll":
                        _fire_ag(L, 0)
                if not is_final and variant != "no_coll":
                    _fire_ag(L, 1)

            def _fire_ag(L, half):
                src = L["ag"][half]
                dst = L["nxt"][half]
                nr = SPLIT if half == 0 else SPLB
                nc.gpsimd.collective_compute(
                    "AllGather", OP.bypass, replica_groups=RG,
                    ins=[src[:, :]],
                    outs=[dst[:]])

            for li, L in enumerate(layers):
                do_layer(li, L)

    nc.compile()
    return nc


_CACHE = {}


def make_in_maps(inputs, pp):
    x = np.asarray(inputs["x"], np.float32)
    row_real = pp["row_real"]
    x_rows = np.zeros((NPT, 128), np.float32)
    x_rows[row_real] = x[pp["node_of_row"][row_real]]
    table0 = (pp["dinv_rows"][:, None] * x_rows).astype(_bf16)
    # per-core own rows in local order
    dloc = pp["dloc"]
    t0own = np.zeros((C, NPCP, 128), np.float32)
    for c in range(C):
        t0own[c, :NPC] = dloc[c, :NPC, None] * x[c * NPC:(c + 1) * NPC]
    t0own = t0own.astype(_bf16)

    vecs = np.zeros((9, 256), np.float32)
    for i, k in enumerate(["b0", "b1", "b2", "g0", "g1", "g2",
                           "beta0", "beta1", "beta2"]):
        v = np.asarray(inputs[k], np.float32)
        vecs[i, :len(v)] = v
    b3col = np.asarray(inputs["b3"], np.float32).reshape(128, 1)

    common = {
        "t0": table0,
        "w0": np.asarray(inputs["W0"]).astype(_bf16),
        "w1": np.asarray(inputs["W1"]).astype(_bf16),
        "w2": np.asarray(inputs["W2"]).astype(_bf16),
        "w3": np.asarray(inputs["W3"]).astype(_bf16),
        "vecs": vecs, "b3col": b3col,
    }
    in_maps = []
    for c in range(C):
        in_maps.append({
            **common,
            "t0own": t0own[c],
            "idx": pack_idx(pp["idx"][c]),
            "smat": np.ascontiguousarray(pp["S"][c]),
            "dinv": np.ascontiguousarray(pp["dinv_blocks"][c]),
            "dfrow": pp["dinv_frow"][c],
            "b3col": b3col,
        })
    return in_maps


def _run(nc, in_maps):
    from concourse.bass_utils import run_bass_kernel_spmd
    return run_bass_kernel_spmd(nc, in_maps, core_ids=list(range(C))).results


def kernel(x, edge_index, W0, b0, W1, b1, W2, b2, W3, b3,
           a0, a1, a2, g0, beta0, g1, beta1, g2, beta2):
    edge_index = np.asarray(edge_index)
    pp = preprocess(edge_index)

    key = "graph"
    if key not in _CACHE:
        _CACHE[key] = build(pp)
    nc = _CACHE[key]

    inputs = dict(x=x, W0=W0, b0=b0, W1=W1, b1=b1, W2=W2, b2=b2, W3=W3,
                  b3=b3, g0=g0, beta0=beta0, g1=g1, beta1=beta1, g2=g2,
                  beta2=beta2)
    in_maps = make_in_maps(inputs, pp)
    res = _run(nc, in_maps)

    out = np.zeros((N, 128), np.float32)
    for c in range(C):
        out[c * NPC:(c + 1) * NPC] = res[c]["out"].T[:NPC]
    return out



# revision 8
# speedup vs baseline: 1.3294x; 1.3294x over previous
"""Trainium2 Bass kernel v3 for nn_AdaptiveGNN (4-layer GCN, N=100000,
E=1600000, dims 128->256->256->256->128), 8-core node-sharded.

v3 vs v2 (trace-driven):
- dma_gather cost is ~2.2us/call nearly independent of size -> CHUNK=2688
  (SWDGE scratch 49152) + windows rebalanced so each (bank, window) run is
  one call: ~100 calls/layer (was ~292).
- edge norm (dinv_src*dinv_dst) baked into the S matrices (bf16) and
  self-loops folded into the gather stream: kills the transposed self-row
  DMAs, the psum+self adds, and all dinv scaling passes.
- fused epilogue: tensor_tensor_reduce (bias add + sum), ACT-engine
  Square+accum (sumsq), Identity(scale=rstd,bias=-mu*rstd), wide g/beta,
  ELU via exp-then-min: 5 DVE + 4 ACT passes (was ~15 DVE).
- AllGather in 3 pieces (13/6/6 banks) fired as their banks complete;
  gather emission runs one bank ahead (w0/w1 of bank b before w2/w3 of
  bank b-1) so next-layer head never head-of-line blocks on the tail
  collective.
"""
import sys
for p in ("/opt/trn_rl_repo",):
    if p not in sys.path:
        sys.path.insert(0, p)
import numpy as np
import ml_dtypes

from concourse import bass, bacc, mybir, tile
from concourse.library_config import mlp
from concourse.masks import make_identity

_bf16 = ml_dtypes.bfloat16

N = 100000
E = 1600000
C = 8
NPC = N // C              # 12500 real nodes per core
NPCP = 12544              # padded rows per core (98 blocks of 128)
NPT = C * NPCP            # 100352 table rows
BANK = 512                # destinations per PSUM bank tile
NBANK = (NPCP + BANK - 1) // BANK   # 25 (last bank 256)
# chunk-major row layout: A | B1 | B2 per-core splits (in padded local rows)
SPLA = 6656               # banks 0..12
SPLB1 = 3072              # banks 13..18
SPLB2 = NPCP - SPLA - SPLB1   # 2816, banks 19..24 (incl 44 pad rows)
CUT_A = SPLA // BANK      # 13
CUT_B1 = (SPLA + SPLB1) // BANK  # 19
GA = C * SPLA             # 53248
GB1 = C * SPLB1           # 24576
GB2 = C * SPLB2           # 22528
WBASE = [0, GA // 2, GA, GA + GB1]       # [0, 26624, 53248, 77824]
WEND = [GA // 2, GA, GA + GB1, NPT]
NW = 4
import os as _os
NQ = 4                    # SWDGE queues
SCRATCH = int(_os.environ.get("KSCRATCH", 16384))
CHUNK = int(_os.environ.get("KCHUNK", 3072))
GBUFS = int(_os.environ.get("KGBUFS", 4))

dt = mybir.dt
F32, BF16, I16 = dt.float32, dt.bfloat16, dt.int16
AX = mybir.AxisListType
OP = mybir.AluOpType
AF = mybir.ActivationFunctionType
LN_EPS = 1e-5
RG = [list(range(C))]


def row_of_node(n):
    c = n // NPC
    l = n % NPC
    return np.where(
        l < SPLA, c * SPLA + l,
        np.where(l < SPLA + SPLB1,
                 GA + c * SPLB1 + (l - SPLA),
                 GA + GB1 + c * SPLB2 + (l - SPLA - SPLB1)))


def preprocess(edge_index: np.ndarray):
    """Slot streams (edges + self loops), S matrices carrying the edge norm,
    uniform chunk metadata shared by all 4 layers."""
    src_e = edge_index[0].astype(np.int64)
    dst_e = edge_index[1].astype(np.int64)

    deg = np.bincount(dst_e, minlength=N).astype(np.int64) + 1
    dinv = (1.0 / np.sqrt(deg.astype(np.float64))).astype(np.float64)

    loops = np.arange(N, dtype=np.int64)
    src0 = np.concatenate([src_e, loops])
    dst0 = np.concatenate([dst_e, loops])
    norm0 = (dinv[src0] * dinv[dst0]).astype(np.float32)

    core_of = dst0 // NPC
    d_loc = dst0 % NPC
    src_r = row_of_node(src0)
    w = ((src_r >= WBASE[1]).astype(np.int64)
         + (src_r >= WBASE[2]) + (src_r >= WBASE[3]))
    idxv = src_r - np.asarray(WBASE)[w]
    assert idxv.max() < 32768
    bank = d_loc // BANK

    nn = np.zeros((C, NBANK, NW), np.int64)
    key = ((core_of * NBANK + bank) * NW + w)
    np.add.at(nn.reshape(-1), key, 1)
    nn_max = nn.max(axis=0)                  # [NBANK, NW]

    order = np.argsort(key * NPCP + d_loc, kind="stable")
    s_srt = idxv[order]
    d_srt = d_loc[order]
    k_srt = key[order]
    n_srt = norm0[order]
    run_starts = np.zeros(C * NBANK * NW + 1, np.int64)
    np.cumsum(np.bincount(k_srt, minlength=C * NBANK * NW),
              out=run_starts[1:])

    # ---- uniform chunk layout (shared across cores) ----
    banks_meta = []
    off16 = 0
    slot_base = np.zeros((NBANK, NW), np.int64)
    for b in range(NBANK):
        nb_d = min(BANK, NPCP - b * BANK)
        chunks = []
        for wi in range(NW):
            m = int(nn_max[b, wi])
            assert m > 0, f"empty (bank,window) ({b},{wi})"
            slot_base[b, wi] = off16 * 16
            pos = 0
            while pos < m:
                size = min(CHUNK, m - pos)
                ngrp = (size + 127) // 128
                groups = [dict(gi=gi, s_off=0, c0=0, ncols=0,
                               first=False, last=False)
                          for gi in range(ngrp)]
                chunks.append(dict(w=wi, off16=off16, nn=size, groups=groups,
                                   slot0=off16 * 16))
                off16 += (size + 15) // 16
                pos += size
        chunks[0]["groups"][0]["first"] = True
        chunks[-1]["groups"][-1]["last"] = True
        banks_meta.append(dict(nb_d=nb_d, chunks=chunks))
    ICOLS = off16

    # ---- per-core slot arrays ----
    TOTS = ICOLS * 16
    idx_all = np.zeros((C, TOTS), np.int16)
    dcol_all = np.full((C, TOTS), -1, np.int64)
    nval_all = np.zeros((C, TOTS), np.float32)
    for c in range(C):
        for b in range(NBANK):
            for wi in range(NW):
                r = (c * NBANK + b) * NW + wi
                lo, hi = run_starts[r], run_starts[r + 1]
                cnt = hi - lo
                base = slot_base[b, wi]
                idx_all[c, base:base + cnt] = s_srt[lo:hi]
                dcol_all[c, base:base + cnt] = d_srt[lo:hi] - b * BANK
                nval_all[c, base:base + cnt] = n_srt[lo:hi]

    # ---- group col ranges (min/max over cores) + S offsets ----
    for b, bm in enumerate(banks_meta):
        nb_d = bm["nb_d"]
        for ch in bm["chunks"]:
            s0 = ch["slot0"]
            for g in ch["groups"]:
                a = s0 + g["gi"] * 128
                e = min(s0 + ch["nn"], a + 128)
                dv = dcol_all[:, a:e]
                valid = dv >= 0
                if g["first"]:
                    g["c0"], g["ncols"] = 0, nb_d
                elif valid.any():
                    g["c0"] = int(dv[valid].min())
                    g["ncols"] = int(dv[valid].max()) - g["c0"] + 1
                else:
                    g["c0"], g["ncols"] = 0, 1
    scols = 0
    for bm in banks_meta:
        scols = (scols + 15) // 16 * 16
        bm["s_off"] = scols
        for ch in bm["chunks"]:
            for g in ch["groups"]:
                g["s_off"] = scols
                scols += g["ncols"]
        bm["s_cols"] = scols - bm["s_off"]
    SCOLS = (scols + 15) // 16 * 16

    S = np.zeros((C, 128, SCOLS), np.float32)
    for b, bm in enumerate(banks_meta):
        for ch in bm["chunks"]:
            s0 = ch["slot0"]
            for g in ch["groups"]:
                a = s0 + g["gi"] * 128
                e = min(s0 + ch["nn"], a + 128)
                dv = dcol_all[:, a:e]
                valid = dv >= 0
                ci, pi = np.nonzero(valid)
                cols = dv[ci, pi] - g["c0"]
                np.add.at(S, (ci, pi, g["s_off"] + cols),
                          nval_all[ci, a + pi])

    return dict(banks=banks_meta, ICOLS=ICOLS, SCOLS=SCOLS,
                idx=idx_all, S=S.astype(_bf16))


def pack_idx(enc_core: np.ndarray) -> np.ndarray:
    """[TOTS] int16 -> [128, TOTS//16], slot i at (i%16, i//16), tiled x8."""
    TOTS = len(enc_core)
    block = enc_core.reshape(TOTS // 16, 16).T
    return np.tile(block, (8, 1)).copy()


def build(pp, variant="full"):
    banks = pp["banks"]
    ICOLS = pp["ICOLS"]
    SCOLS = pp["SCOLS"]
    max_scols = max(bm["s_cols"] for bm in banks)
    max_scols = (max_scols + 15) // 16 * 16

    nc = bacc.Bacc("TRN2", target_bir_lowering=False, debug=False,
                   num_devices=C, num_swdge_queues=NQ,
                   dynamic_dma_scratch_size=SCRATCH)

    t0 = nc.dram_tensor("t0", [NPT, 128], BF16, kind="ExternalInput")
    idx = nc.dram_tensor("idx", [128, ICOLS], I16, kind="ExternalInput")
    s_in = nc.dram_tensor("smat", [128, SCOLS], BF16, kind="ExternalInput")
    b3col_in = nc.dram_tensor("b3col", [128, 1], F32, kind="ExternalInput")
    w0_in = nc.dram_tensor("w0", [128, 256], BF16, kind="ExternalInput")
    w1_in = nc.dram_tensor("w1", [256, 256], BF16, kind="ExternalInput")
    w2_in = nc.dram_tensor("w2", [256, 256], BF16, kind="ExternalInput")
    w3_in = nc.dram_tensor("w3", [256, 128], BF16, kind="ExternalInput")
    # rows 0..2 = b0,b1,b2; 3..5 = g0,g1,g2; 6..8 = beta0..2
    vecs = nc.dram_tensor("vecs", [9, 256], F32, kind="ExternalInput")
    out = nc.dram_tensor("out", [128, NPCP], F32, kind="ExternalOutput")

    with tile.TileContext(nc) as tc:
        nc.gpsimd.load_library(mlp)
        with (tc.tile_pool(name="dram", bufs=1, space="DRAM") as dr,
              tc.tile_pool(name="const", bufs=1) as cp,
              tc.tile_pool(name="sp", bufs=3) as spl,
              tc.tile_pool(name="ap", bufs=2) as apl,
              tc.tile_pool(name="ep", bufs=2) as ep,
              tc.tile_pool(name="hp", bufs=2) as hp,
              tc.tile_pool(name="pp", bufs=2, space="PSUM") as pps,
              tc.tile_pool(name="pw", bufs=2, space="PSUM") as pwp,
              tc.tile_pool(name="pz", bufs=1, space="PSUM") as pzp):

            # per-boundary local chunks + shared all-gathered tables
            agA1 = dr.tile([SPLA, 256], BF16)
            agB11 = dr.tile([SPLB1, 256], BF16)
            agB21 = dr.tile([SPLB2, 256], BF16)
            tA1 = dr.tile([GA, 256], BF16, addr_space="Shared")
            tB11 = dr.tile([GB1, 256], BF16, addr_space="Shared")
            tB21 = dr.tile([GB2, 256], BF16, addr_space="Shared")
            agA2 = dr.tile([SPLA, 256], BF16)
            agB12 = dr.tile([SPLB1, 256], BF16)
            agB22 = dr.tile([SPLB2, 256], BF16)
            tA2 = dr.tile([GA, 256], BF16, addr_space="Shared")
            tB12 = dr.tile([GB1, 256], BF16, addr_space="Shared")
            tB22 = dr.tile([GB2, 256], BF16, addr_space="Shared")
            agAz = dr.tile([SPLA, 128], BF16)
            agB1z = dr.tile([SPLB1, 128], BF16)
            agB2z = dr.tile([SPLB2, 128], BF16)
            tAz = dr.tile([GA, 128], BF16, addr_space="Shared")
            tB1z = dr.tile([GB1, 128], BF16, addr_space="Shared")
            tB2z = dr.tile([GB2, 128], BF16, addr_space="Shared")

            # ---- constants ----
            idx_t = cp.tile([128, ICOLS], I16)
            nc.sync.dma_start(out=idx_t[:], in_=idx[:])
            b3c = cp.tile([128, 1], F32)
            nc.sync.dma_start(out=b3c[:], in_=b3col_in[:])
            w0_sb = cp.tile([128, 1, 256], BF16)
            nc.sync.dma_start(out=w0_sb[:],
                              in_=w0_in[:].rearrange("(j p) o -> p j o", p=128))
            w1_sb = cp.tile([128, 2, 256], BF16)
            nc.sync.dma_start(out=w1_sb[:],
                              in_=w1_in[:].rearrange("(j p) o -> p j o", p=128))
            w2_sb = cp.tile([128, 2, 256], BF16)
            nc.sync.dma_start(out=w2_sb[:],
                              in_=w2_in[:].rearrange("(j p) o -> p j o", p=128))
            w3_sb = cp.tile([128, 2, 128], BF16)
            nc.sync.dma_start(out=w3_sb[:],
                              in_=w3_in[:].rearrange("(j p) o -> p j o", p=128))

            vec_row = cp.tile([1, 9 * 256], F32)
            nc.sync.dma_start(out=vec_row[:], in_=vecs[:].flatten().unsqueeze(0))
            bcast = cp.tile([128, 9, 256], F32)
            for r in range(9):
                nc.gpsimd.partition_broadcast(
                    out_ap=bcast[:, r, :],
                    in_ap=vec_row[:, r * 256:(r + 1) * 256])

            ident = cp.tile([128, 128], BF16)
            make_identity(nc, ident[:])
            eps_t = cp.tile([128, 1], F32)
            nc.vector.memset(eps_t[:], LN_EPS)

            gbig = cp.tile([128, GBUFS, ((CHUNK + 127) // 128) * 256], BF16)
            nc.vector.memset(gbig[:], 0.0)

            def vbc(r, nb, d=256):
                return bcast[:, r:r + 1, :d].to_broadcast([128, nb, d])

            layers = [
                dict(wins=[(t0, WBASE[i]) for i in range(NW)], Din=128,
                     w=w0_sb, bias=0, g=3, beta=6,
                     ag=(agA1, agB11, agB21), nxt=(tA1, tB11, tB21)),
                dict(wins=[(tA1, 0), (tA1, GA // 2), (tB11, 0), (tB21, 0)],
                     Din=256, w=w1_sb, bias=1, g=4, beta=7,
                     ag=(agA2, agB12, agB22), nxt=(tA2, tB12, tB22)),
                dict(wins=[(tA2, 0), (tA2, GA // 2), (tB12, 0), (tB22, 0)],
                     Din=256, w=w2_sb, bias=2, g=5, beta=8,
                     ag=(agAz, agB1z, agB2z), nxt=(tAz, tB1z, tB2z),
                     with_z=True),
                dict(wins=[(tAz, 0), (tAz, GA // 2), (tB1z, 0), (tB2z, 0)],
                     Din=128, w=None),
            ]

            grot = [0]

            def _fire_ag(L, piece):
                src = L["ag"][piece]
                dst = L["nxt"][piece]
                nc.gpsimd.collective_compute(
                    "AllGather", OP.bypass, replica_groups=RG,
                    ins=[src[:, :]], outs=[dst[:]])

            def emit_gathers(li, L, b, wset, pts):
                """gathers + aggregation matmuls of bank b for windows wset"""
                Din = L["Din"]
                J = Din // 128
                bm = banks[b]
                soff0 = bm["s_off"]
                s_t = bank_s[b % 3]
                for ch in bm["chunks"]:
                    if ch["w"] not in wset:
                        continue
                    nn_c = ch["nn"]
                    ngr = (nn_c + 127) // 128
                    gslot = grot[0] % GBUFS
                    grot[0] += 1
                    gt = gbig[:, gslot, :ngr * Din].rearrange(
                        "p (n e) -> p n e", e=Din)
                    wt, woff = L["wins"][ch["w"]]
                    nc.gpsimd.dma_gather(
                        out_ap=gt,
                        in_ap=wt[woff:, :],
                        idxs_ap=idx_t[:, ch["off16"]:
                                      ch["off16"] + (nn_c + 15) // 16],
                        num_idxs=nn_c, num_idxs_reg=nn_c, elem_size=Din,
                        transpose=False, queue_num=grot[0] % NQ,
                        single_packet=(Din == 128),
                    )
                    if variant == "gather_only":
                        continue
                    for g in ch["groups"]:
                        rel = g["s_off"] - soff0
                        for j in range(J):
                            nc.tensor.matmul(
                                out=pts[j][:, g["c0"]:g["c0"] + g["ncols"]],
                                lhsT=gbig[:, gslot,
                                          g["gi"] * Din + j * 128:
                                          g["gi"] * Din + j * 128 + 128],
                                rhs=s_t[:, rel:rel + g["ncols"]],
                                start=g["first"], stop=g["last"],
                                skip_group_check=True)

            def emit_epilogue(li, L, b, pts):
                Din = L["Din"]
                J = Din // 128
                bm = banks[b]
                nb_d = bm["nb_d"]
                ncr = nb_d // 128
                is_final = L["w"] is None

                if is_final:
                    ob = ep.tile([128, BANK], F32, tag="ob", name=f"ob_{b}")
                    nc.vector.tensor_scalar(
                        out=ob[:, :nb_d], in0=pts[0][:, :nb_d],
                        scalar1=b3c[:, :1], scalar2=None, op0=OP.add)
                    nc.sync.dma_start(
                        out=out[:, b * BANK:b * BANK + nb_d],
                        in_=ob[:, :nb_d])
                    return

                # PSUM -> SBUF bf16 for the W matmul lhsT (ACT engine)
                asb = apl.tile([128, J, BANK], BF16, tag="asb",
                               name=f"as_{li}_{b}")
                for j in range(J):
                    nc.scalar.copy(out=asb[:, j, :nb_d], in_=pts[j][:, :nb_d])

                cs = ep.tile([128, 4, 256], F32, tag="cs", name=f"cs_{li}_{b}")
                Ew = ep.tile([128, 4, 256], F32, tag="Ew", name=f"E_{li}_{b}")
                hm = ep.tile([128, 4, 256], F32, tag="hm", name=f"hm_{li}_{b}")
                sq2 = ep.tile([128, 256], F32, tag="sq2", name=f"sq_{li}_{b}")
                st = ep.tile([128, 6, 4], F32, tag="st", name=f"st_{li}_{b}")
                sums, sumsqs = st[:, 0, :], st[:, 1, :]
                mu, var = st[:, 2, :], st[:, 3, :]
                rstd, nmr = st[:, 4, :], st[:, 5, :]

                for k in range(ncr):
                    pwt = pwp.tile([128, 256], F32, tag="pw",
                                   name=f"pw_{li}_{b}_{k}", space="PSUM")
                    for j in range(J):
                        nc.tensor.matmul(
                            out=pwt[:],
                            lhsT=asb[:, j, k * 128:(k + 1) * 128],
                            rhs=L["w"][:, j, :],
                            start=(j == 0), stop=(j == J - 1))
                    with nc.allow_low_precision("LN stats"):
                        nc.vector.tensor_tensor_reduce(
                            out=cs[:, k, :], in0=pwt[:],
                            in1=bcast[:, L["bias"], :], scale=1.0, scalar=0.0,
                            op0=OP.add, op1=OP.add,
                            accum_out=sums[:, k:k + 1])
                        nc.scalar.activation(
                            out=sq2[:], in_=cs[:, k, :], func=AF.Square,
                            accum_out=sumsqs[:, k:k + 1])

                # batched per-bank stats [128, ncr]
                nc.vector.tensor_scalar(
                    out=mu[:, :ncr], in0=sums[:, :ncr],
                    scalar1=1.0 / 256.0, scalar2=None, op0=OP.mult)
                nc.vector.tensor_tensor(out=var[:, :ncr], in0=mu[:, :ncr],
                                        in1=mu[:, :ncr], op=OP.mult)
                nc.vector.scalar_tensor_tensor(
                    out=var[:, :ncr], in0=sumsqs[:, :ncr],
                    scalar=1.0 / 256.0, op0=OP.mult,
                    in1=var[:, :ncr], op1=OP.subtract)
                nc.scalar.activation(out=var[:, :ncr], in_=var[:, :ncr],
                                     func=AF.Sqrt, bias=eps_t[:, :1])
                nc.vector.reciprocal(out=rstd[:, :ncr], in_=var[:, :ncr])
                nc.vector.scalar_tensor_tensor(
                    out=nmr[:, :ncr], in0=mu[:, :ncr], scalar=-1.0,
                    op0=OP.mult, in1=rstd[:, :ncr], op1=OP.mult)

                # normalize (ACT), then g/beta + ELU (wide DVE)
                for k in range(ncr):
                    nc.scalar.activation(
                        out=cs[:, k, :], in_=cs[:, k, :], func=AF.Identity,
                        bias=nmr[:, k:k + 1], scale=rstd[:, k:k + 1])
                nc.vector.tensor_tensor(out=cs[:, :ncr, :], in0=cs[:, :ncr, :],
                                        in1=vbc(L["g"], ncr), op=OP.mult)
                nc.vector.tensor_tensor(out=cs[:, :ncr, :], in0=cs[:, :ncr, :],
                                        in1=vbc(L["beta"], ncr), op=OP.add)
                nc.scalar.activation(out=Ew[:, :ncr, :], in_=cs[:, :ncr, :],
                                     func=AF.Exp)
                nc.vector.tensor_scalar(
                    out=hm[:, :ncr, :], in0=cs[:, :ncr, :],
                    scalar1=0.0, scalar2=-1.0, op0=OP.max, op1=OP.add)
                hh = hp.tile([128, 4, 256], BF16, tag="hh",
                             name=f"h_{li}_{b}")
                nc.vector.scalar_tensor_tensor(
                    out=hh[:, :ncr, :], in0=Ew[:, :ncr, :], scalar=1.0,
                    op0=OP.min, in1=hm[:, :ncr, :], op1=OP.add)

                # destination rows of this bank in the A|B1|B2 local chunks
                if b < CUT_A:
                    agt, arow = L["ag"][0], b * BANK
                elif b < CUT_B1:
                    agt, arow = L["ag"][1], b * BANK - SPLA
                else:
                    agt, arow = L["ag"][2], b * BANK - SPLA - SPLB1

                if not L.get("with_z"):
                    nc.sync.dma_start(
                        out=agt[arow:arow + nb_d, :]
                            .rearrange("(nb p) d -> p nb d", p=128),
                        in_=hh[:, :ncr, :])
                else:
                    # z = hh @ W3  (transpose each 128x128 block via PE)
                    zc = hp.tile([128, 4, 128], BF16, tag="zc",
                                 name=f"zc_{b}")
                    for k in range(ncr):
                        h3T = hp.tile([128, 2, 128], BF16, tag="h3T",
                                      name=f"h3T_{b}_{k}")
                        for j in range(2):
                            pt2 = pzp.tile([128, 128], BF16, tag="ptz",
                                           name=f"ptz_{b}_{k}_{j}",
                                           space="PSUM")
                            nc.tensor.transpose(
                                out=pt2[:],
                                in_=hh[:, k, j * 128:(j + 1) * 128],
                                identity=ident[:])
                            nc.vector.tensor_copy(out=h3T[:, j, :],
                                                  in_=pt2[:])
                        pz = pzp.tile([128, 128], F32, tag="pzz",
                                      name=f"pz_{b}_{k}", space="PSUM")
                        for j in range(2):
                            nc.tensor.matmul(
                                out=pz[:], lhsT=h3T[:, j, :],
                                rhs=w3_sb[:, j, :],
                                start=(j == 0), stop=(j == 1))
                        nc.vector.tensor_copy(out=zc[:, k, :], in_=pz[:])
                    nc.sync.dma_start(
                        out=agt[arow:arow + nb_d, :]
                            .rearrange("(nb p) d -> p nb d", p=128),
                        in_=zc[:, :ncr, :])

            bank_s = {}

            def do_layer(li, L):
                Din = L["Din"]
                J = Din // 128
                is_final = L["w"] is None
                pts_of = {}

                def start_bank(b):
                    bm = banks[b]
                    s_t = spl.tile([128, max_scols], BF16, tag="S",
                                   name=f"S_{li}_{b}")
                    bank_s[b % 3] = s_t
                    sc16 = (bm["s_cols"] + 15) // 16 * 16
                    nc.sync.dma_start(
                        out=s_t[:, :sc16],
                        in_=s_in[:, bm["s_off"]:bm["s_off"] + sc16])
                    pts = [pps.tile([128, BANK], F32, tag=f"agg{j}",
                                    name=f"pt{j}_{li}_{b}", space="PSUM")
                           for j in range(J)]
                    pts_of[b] = pts
                    emit_gathers(li, L, b, (0, 1), pts)

                def finish_bank(b):
                    emit_gathers(li, L, b, (2, 3), pts_of[b])
                    if variant != "gather_only":
                        emit_epilogue(li, L, b, pts_of[b])
                    del pts_of[b]
                    if is_final or variant == "no_coll" or \
                            variant == "gather_only":
                        return
                    if b == CUT_A - 1:
                        _fire_ag(L, 0)
                    elif b == CUT_B1 - 1:
                        _fire_ag(L, 1)
                    elif b == NBANK - 1:
                        _fire_ag(L, 2)

                start_bank(0)
                for b in range(1, NBANK):
                    start_bank(b)
                    finish_bank(b - 1)
                finish_bank(NBANK - 1)

            for li, L in enumerate(layers):
                do_layer(li, L)

    # The tile scheduler may reorder gathers, so emission-order queue
    # rotation can desync from the DMASW lane rotation (sem lanes are
    # assigned in scheduled order and each sem must stay on one queue).
    # Re-derive queue_num from the assigned DMASW lane.  Must happen
    # BEFORE nc.compile() so the ISA encoding picks up the new value.
    import re as _re
    nfix = 0
    for blk in nc.m.functions[0].blocks:
        for inst in blk.instructions:
            if isinstance(inst, mybir.InstDMAGatherAnt):
                si = inst.sync_info
                if si is None:
                    continue
                for u in si.on_update:
                    m = _re.search(r"DMASW(\d+)_", str(u))
                    if m:
                        inst.queue_num = int(m.group(1)) % NQ
                        nfix += 1
                        break
    assert nfix > 0, "no gather queue_nums rewritten - sem naming changed?"
    nc.compile()
    return nc


_CACHE = {}


def make_in_maps(inputs, pp):
    x = np.asarray(inputs["x"], np.float32)
    rows = np.arange(NPT)
    # invert row_of_node: which node occupies each table row (pads -> 0)
    node_of_row = np.zeros(NPT, np.int64)
    row_real = np.zeros(NPT, bool)
    nodes = np.arange(N, dtype=np.int64)
    r = row_of_node(nodes)
    node_of_row[r] = nodes
    row_real[r] = True
    x_rows = np.zeros((NPT, 128), np.float32)
    x_rows[row_real] = x[node_of_row[row_real]]
    table0 = x_rows.astype(_bf16)

    vecs = np.zeros((9, 256), np.float32)
    for i, k in enumerate(["b0", "b1", "b2", "g0", "g1", "g2",
                           "beta0", "beta1", "beta2"]):
        v = np.asarray(inputs[k], np.float32)
        vecs[i, :len(v)] = v
    b3col = np.asarray(inputs["b3"], np.float32).reshape(128, 1)

    common = {
        "t0": table0,
        "w0": np.asarray(inputs["W0"]).astype(_bf16),
        "w1": np.asarray(inputs["W1"]).astype(_bf16),
        "w2": np.asarray(inputs["W2"]).astype(_bf16),
        "w3": np.asarray(inputs["W3"]).astype(_bf16),
        "vecs": vecs, "b3col": b3col,
    }
    in_maps = []
    for c in range(C):
        in_maps.append({
            **common,
            "idx": pack_idx(pp["idx"][c]),
            "smat": np.ascontiguousarray(pp["S"][c]),
        })
    return in_maps


def _run(nc, in_maps):
    from concourse.bass_utils import run_bass_kernel_spmd
    return run_bass_kernel_spmd(nc, in_maps, core_ids=list(range(C))).results


def kernel(x, edge_index, W0, b0, W1, b1, W2, b2, W3, b3,
           a0, a1, a2, g0, beta0, g1, beta1, g2, beta2):
    edge_index = np.asarray(edge_index)
    pp = preprocess(edge_index)

    key = "graph"
    if key not in _CACHE:
        _CACHE[key] = build(pp)
    nc = _CACHE[key]

    inputs = dict(x=x, W0=W0, b0=b0, W1=W1, b1=b1, W2=W2, b2=b2, W3=W3,
                  b3=b3, g0=g0, beta0=beta0, g1=g1, beta1=beta1, g2=g2,
                  beta2=beta2)
    in_maps = make_in_maps(inputs, pp)
    res = _run(nc, in_maps)

    out = np.zeros((N, 128), np.float32)
    for c in range(C):
        out[c * NPC:(c + 1) * NPC] = res[c]["out"].T[:NPC]
    return out
